# revision 40
# baseline (speedup 1.0000x reference)
# Trainium2 Bass kernel for nn_Lowrank_Spattention (sparse_attention).
#
# Reference math (per batch b, n=8192 tokens, f=256 features, h=4 heads,
# r=64 latent ranks, d=64 head dim):
#   q    = z @ Wq + bq                    (n, h*d)
#   attn = einsum(q, K)/sqrt(d)           (n, h*r)   == z @ M + ab
#            where M[:, h*r+j] = (Wq_h @ K_h^T)/8,  ab = bq @ K^T/8
#   xv   = x @ Wv + bv                    (n, h*d)
#   pooled = softmax_r(attn)^T-pool of xv (n-pool)   (r, h, d)
#   v    = softmax_n(attn) @ pooled       (n, h, d)
#   out  = sig(alpha)*xv + sig(beta)*v
#
# Kernel strategy (one NeuronCore per batch element, 8 cores, no collectives):
#   The host ships z^T (fp8), x packed (fp8), x^T (fp16), so the device
#   never transposes z or x on the PE.  The v-path's contribution to the
#   output is scaled by sig(beta)~=0.01 and pooled averages 8192 rows, so
#   its relative error is damped ~1e2-1e4x: everything E-side runs fp8
#   (DoubleRow matmuls, 2x PE throughput).  The xv-path (the dominant term)
#   is fp16 x^T against fp16 sWv.  Output is stored fp16, upcast on host.
#   Total HBM traffic 12 MB/core (was 24).
#
#   Pass A (per 128-row chunk, 4-stage pipeline over quads of chunks):
#     attn = DR(zT chunk, M)               (PE fp8 DoubleRow, one op/chunk)
#     E    = exp(attn)                     (Act->fp16; attn~N(0,1), no max sub)
#     E^T  = PE-transpose of E             (PE->PSUM, DVE copies to SBUF)
#     rs/64 = E^T @ head_indicator/64      (PE, free-dim 4: ~free)
#     rcp  = 64/rs (DVE), rs/64 -> x_s aux cols (DVE, fp8)
#     Eh'  = E * rcp  (Pool, fp8; = 64*E/rowsum)
#     G   += DR(Eh' 2-chunk, [x|aux] 2-chunk)  (PE fp8 DoubleRow, accumulate)
#   Finalize (tiny): pooled = G[:, :256] @ Wv + esum*bv;
#     PS = sig(beta)/64 * pooled / colsum, block-diagonal (fp16).
#     (colsum recovery: G[(h,r), 256+h] = sum_n Eh'*rs_fp8/64 ~= sum_n E.)
#   Pass B (per chunk): out = lhsT(xT chunk) @ sWv + lhsT(E^T) @ PS_bd
#     pure matmuls, one PSUM group per chunk, fp16 store.

import math
import os

import numpy as np

import concourse.bass as bass
import concourse.mybir as mybir
import concourse.tile as tile
from concourse import bacc

B, N, DIM = 8, 8192, 256
HEAD, RANK, HDIM = 4, 64, 64
NCORES = 8
CHUNK = 128                 # rows per compute chunk
NCHUNK = N // CHUNK         # 64
XW = DIM + 8                # x_s row width: 256 x cols + [rs0..3 | 1 | 1 1 1]
SUPER = 4                   # chunks per DMA super-chunk
NSUPER = NCHUNK // SUPER    # 8
NQ = NCHUNK // 4            # quads

F32 = mybir.dt.float32
F32R = mybir.dt.float32r
F16 = mybir.dt.float16
F8 = mybir.dt.float8e4
DR = mybir.MatmulPerfMode.DoubleRow
Exp = mybir.ActivationFunctionType.Exp


def build_body(tc, outs, ins):
    """Emit the per-core program.  outs/ins are dicts of bass.APs."""
    nc = tc.nc
    zt, xt, xn = ins["zt"], ins["xt"], ins["xn"]
    out = outs["out"]
    has_ab = ins.get("ab_row") is not None
    has_bias = bool(ins.get("has_bias", True))

    with (
        tc.tile_pool(name="consts", bufs=1) as consts,
        tc.tile_pool(name="resident", bufs=1) as resident,
    ):
        # ---- constants ----
        ident_h = consts.tile([128, 128], F16)
        nc.gpsimd.memset(ident_h, 0.0)
        nc.gpsimd.affine_select(
            out=ident_h, in_=ident_h,
            compare_op=mybir.AluOpType.not_equal, fill=1.0,
            base=0, pattern=[[-1, 128]], channel_multiplier=1,
        )
        # head indicator / 64 for PE rowsums: hind[p, t, h] = (head(t,p)==h)/64
        hind = consts.tile([128, 2, HEAD], F16)
        nc.gpsimd.memset(hind, 0.0)
        for t in range(2):
            nc.gpsimd.memset(hind[0:64, t, 2 * t : 2 * t + 1], 1.0 / 64)
            nc.gpsimd.memset(hind[64:128, t, 2 * t + 1 : 2 * t + 2], 1.0 / 64)

        mq_s = consts.tile([128, 2, DIM], F8)
        nc.sync.dma_start(out=mq_s, in_=ins["mq"].rearrange("(t p) n -> p t n", p=128))
        swv_s = consts.tile([128, 2, DIM], F16)
        nc.sync.dma_start(out=swv_s, in_=ins["swv"].rearrange("(t p) n -> p t n", p=128))
        # finalize-only consts: tiles now, DMAs issued after the pass-A loop
        # so they don't delay the first zt/xt/xn loads on the SP queue
        wv_s = consts.tile([128, 2, DIM], F16)
        sbcol_s = consts.tile([128, 2], F32)
        # broadcast rows across partitions (SWDGE replication)
        bvp_bc = consts.tile([128, DIM], F32)
        nc.gpsimd.dma_start(out=bvp_bc, in_=ins["bv_row"].to_broadcast([128, DIM]))
        biasout_bc = consts.tile([128, DIM], F32)
        nc.gpsimd.dma_start(
            out=biasout_bc, in_=ins["biasout_row"].to_broadcast([128, DIM])
        )
        if has_ab:
            ones_row = consts.tile([1, 128], F16)
            nc.vector.memset(ones_row, 1.0)
            ab_s = consts.tile([1, DIM], F16)
            nc.sync.dma_start(out=ab_s, in_=ins["ab_row"])

        # ---- residents ----
        zt_s = resident.tile([128, 2, N], F8)       # z^T (feat k-tile, n)
        xt_s = resident.tile([128, 2, N], F16)      # x^T
        x_s = resident.tile([128, NCHUNK, XW], F8)  # x natural + aux cols
        et_s = resident.tile([128, NCHUNK, 2, 128], F16)  # E^T (hr-part, rows)
        xv_s = resident.tile([128, NCHUNK, DIM], F16)  # sig(a)*(x@Wv) (+bias)
        psbd = resident.tile([128, 2, 128], F16)    # block-diag PS (pass-B rhs)
        nc.gpsimd.memset(psbd, 0.0)

        # DRAM views (chunk-major row mapping: row = c*128 + p)
        zt_m = zt.rearrange("t p n -> p t n")
        xt_m = xt.rearrange("t p n -> p t n")
        o_m = out.rearrange("(c p) f -> p c f", p=128)

        with (
            tc.tile_pool(name="g_psum", bufs=1, space="PSUM") as gp,
            tc.tile_pool(name="fin_sbuf", bufs=1) as fin,
        ):
            g0 = gp.tile([128, XW], F32, tag="g0")
            g1 = gp.tile([128, XW], F32, tag="g1")

            # ================= Pass A =================
            pa_ctx = (
                tc.tile_pool(name="pa_sbuf", bufs=4),
                tc.tile_pool(name="pa_psum", bufs=1, space="PSUM"),
                tc.tile_pool(name="pa_psum1", bufs=1, space="PSUM"),
            )
            pa = pa_ctx[0].__enter__()
            pap = pa_ctx[1].__enter__()
            pap1 = pa_ctx[2].__enter__()
            e_alls, rcps = {}, {}

            def load_sc(sc):
                cols = slice(sc * SUPER * CHUNK, (sc + 1) * SUPER * CHUNK)
                nc.sync.dma_start(out=zt_s[:, :, cols], in_=zt_m[:, :, cols])
                nc.sync.dma_start(out=xt_s[:, :, cols], in_=xt_m[:, :, cols])
                nc.sync.dma_start(
                    out=x_s[:, sc * SUPER : (sc + 1) * SUPER, :],
                    in_=xn[:, sc * SUPER : (sc + 1) * SUPER, :],
                )

            load_sc(0)

            QPS = SUPER // 4   # quads per super-chunk

            def st_attn(i):
                c = 4 * i
                if (i + 1) % QPS == 0 and (i + 1) // QPS < NSUPER:
                    load_sc((i + 1) // QPS)
                attn_ps = pap.tile([128, 4, DIM], F32, tag="attn_ps")
                for j in range(4):
                    cc = slice((c + j) * CHUNK, (c + j + 1) * CHUNK)
                    nc.tensor.matmul(
                        attn_ps[:, j, :], zt_s[:, :, cc], mq_s,
                        start=True, stop=not has_ab, perf_mode=DR,
                    )
                    if has_ab:
                        nc.tensor.matmul(
                            attn_ps[:, j, :], ones_row, ab_s,
                            start=False, stop=True,
                        )
                e_all = pa.tile([128, 4, DIM], F16, tag="e_all")
                e_alls[i] = e_all
                nc.scalar.activation(e_all, attn_ps, Exp)
                # xv = lhsT(xT chunk) @ sWv while the PE would otherwise idle
                # (pass A is DMA-bound); copies split DVE/Pool
                xv_ps = pap.tile([128, 4, DIM], F32, tag="xv_ps")
                for j in range(4):
                    cc = slice((c + j) * CHUNK, (c + j + 1) * CHUNK)
                    nc.tensor.matmul(
                        xv_ps[:, j, :], xt_s[:, 0, cc], swv_s[:, 0, :],
                        start=True, stop=False,
                    )
                    nc.tensor.matmul(
                        xv_ps[:, j, :], xt_s[:, 1, cc], swv_s[:, 1, :],
                        start=False, stop=True,
                    )
                if has_bias:
                    bias_bc2 = bass.AP(
                        tensor=biasout_bc.tensor,
                        offset=biasout_bc.offset,
                        ap=[biasout_bc.ap[0], [0, 2], [1, DIM]],
                    )
                    nc.vector.tensor_add(
                        xv_s[:, c : c + 2, :], xv_ps[:, 0:2, :], bias_bc2
                    )
                    nc.vector.tensor_add(
                        xv_s[:, c + 2 : c + 4, :], xv_ps[:, 2:4, :], bias_bc2
                    )
                else:
                    nc.vector.tensor_copy(xv_s[:, c : c + 2, :], xv_ps[:, 0:2, :])
                    nc.scalar.copy(xv_s[:, c + 2 : c + 4, :], xv_ps[:, 2:4, :])

            def st_trans(i):
                c = 4 * i
                et_ps = pap1.tile([128, 4, 2, 128], F16, tag="et_ps")
                for j in range(4):
                    for kt in range(2):
                        nc.tensor.transpose(
                            et_ps[:, j, kt, :],
                            e_alls[i][:, j, kt * 128 : (kt + 1) * 128],
                            ident_h,
                        )
                nc.vector.tensor_copy(et_s[:, c : c + 4, :, :], et_ps)

            def st_rs(i):
                c = 4 * i
                rs_ps = pap1.tile([128, 4, HEAD], F32, tag="rs_ps")
                for j in range(4):
                    for t in range(2):
                        nc.tensor.matmul(
                            rs_ps[:, j, :],
                            et_s[:, c + j, t, :], hind[:, t, :],
                            start=(t == 0), stop=(t == 1),
                        )
                aux = bass.AP(
                    tensor=x_s.tensor,
                    offset=x_s.offset + c * XW + DIM,
                    ap=[x_s.ap[0], [XW, 4], [1, 4]],
                )
                with nc.allow_low_precision(reason="damped v-path"):
                    nc.vector.tensor_copy(aux, rs_ps)
                rcp = pa.tile([128, 4, HEAD], F32, tag="rcp")
                rcps[i] = rcp
                nc.vector.reciprocal(rcp, rs_ps)
                eh = pa.tile([128, 4, HEAD, RANK], F8, tag="eh")
                rcp_bc = bass.AP(
                    tensor=rcp.tensor,
                    offset=rcp.offset,
                    ap=[rcp.ap[0], [4, 4], [1, 4], [0, RANK]],
                )
                with nc.allow_low_precision(reason="damped v-path"):
                    nc.gpsimd.tensor_tensor(
                        out=eh,
                        in0=e_alls[i].rearrange("p c (h r) -> p c h r", h=HEAD),
                        in1=rcp_bc,
                        op=mybir.AluOpType.mult,
                    )
                return eh

            ehs = {}

            def st_g(i):
                c = 4 * i
                eh2 = ehs.pop(i).rearrange("p c h r -> p c (h r)")
                for j in (0, 2):
                    for gi, g in enumerate((g0, g1)):
                        nc.tensor.matmul(
                            g[:, 0:XW],
                            eh2[:, j : j + 2, gi * 128 : (gi + 1) * 128],
                            x_s[:, c + j : c + j + 2, :],
                            start=(c + j == 0),
                            stop=(c + j == NCHUNK - 2),
                            perf_mode=DR,
                        )

            for i in range(NQ + 3):
                if i == 3:
                    nc.sync.dma_start(
                        out=wv_s, in_=ins["wv"].rearrange("(t p) n -> p t n", p=128)
                    )
                    nc.sync.dma_start(out=sbcol_s, in_=ins["sbcol"])
                if i < NQ:
                    st_attn(i)
                if 1 <= i < NQ + 1:
                    st_trans(i - 1)
                if 2 <= i < NQ + 2:
                    ehs[i - 2] = st_rs(i - 2)
                if 3 <= i:
                    st_g(i - 3)
                    e_alls.pop(i - 3)
                    rcps.pop(i - 3)
            pa_ctx[2].__exit__(None, None, None)
            pa_ctx[1].__exit__(None, None, None)
            pa_ctx[0].__exit__(None, None, None)

            # ================= Finalize =================
            finp_ctx = tc.tile_pool(name="fin_psum", bufs=1, space="PSUM")
            finp = finp_ctx.__enter__()
            for gi, g in enumerate((g0, g1)):
                gs = fin.tile([128, XW], F16, tag=f"gs{gi}")
                nc.vector.tensor_copy(gs, g)
                gt_ps = finp.tile([128, 2, 128], F16, tag="gt_ps")
                for kt in range(2):
                    nc.tensor.transpose(
                        gt_ps[:, kt, :],
                        gs[:, kt * 128 : (kt + 1) * 128],
                        ident_h,
                    )
                gt = fin.tile([128, 2, 128], F16, tag="gt")
                nc.scalar.copy(gt, gt_ps)
                p_ps = finp.tile([128, 128], F32, tag="p_ps")
                for kt in range(2):
                    nc.tensor.matmul(
                        p_ps,
                        gt[:, kt, :],
                        wv_s[:, kt, gi * 128 : (gi + 1) * 128],
                        start=(kt == 0), stop=(kt == 1),
                    )
                # pooled = p_ps + esum * bv   (esum at aux col 260)
                pool_s = fin.tile([128, 128], F32, tag=f"pool_s{gi}")
                nc.vector.scalar_tensor_tensor(
                    out=pool_s,
                    in0=bvp_bc[:, gi * 128 : (gi + 1) * 128],
                    scalar=gs[:, DIM + 4 : DIM + 5],
                    in1=p_ps,
                    op0=mybir.AluOpType.mult,
                    op1=mybir.AluOpType.add,
                )
                # colsum (col 256+h for head h; even head rows 0:64, odd 64:128)
                cs = fin.tile([128, 1], F32, tag=f"cs{gi}")
                h0, h1 = 2 * gi, 2 * gi + 1
                nc.vector.tensor_copy(cs[0:64, :], gs[0:64, DIM + h0 : DIM + h0 + 1])
                nc.vector.tensor_copy(
                    cs[64:128, :], gs[64:128, DIM + h1 : DIM + h1 + 1]
                )
                rcs = fin.tile([128, 1], F32, tag=f"rcs{gi}")
                nc.vector.reciprocal(rcs, cs)
                nc.vector.tensor_mul(rcs, rcs, sbcol_s[:, gi : gi + 1])
                # PS block-diag (fp16): rows = this pair's (h even r | h odd r)
                nc.vector.tensor_scalar_mul(
                    psbd[0:64, gi, 0:64], pool_s[0:64, 0:64], rcs[0:64, :]
                )
                nc.vector.tensor_scalar_mul(
                    psbd[64:128, gi, 64:128], pool_s[64:128, 64:128], rcs[64:128, :]
                )

            finp_ctx.__exit__(None, None, None)

        # ================= Pass B =================
        # out = xv_s + lhsT(E^T) @ PS_bd; v-matmuls only, add + fp16 store
        with (
            tc.tile_pool(name="pb_sbuf", bufs=2) as pb,
            tc.tile_pool(name="pb_psum", bufs=4, space="PSUM") as pbp,
        ):
            for sc in range(NSUPER):
                ostage = pb.tile([128, SUPER, DIM], F16, tag="ostage")
                for q in range(SUPER // 4):
                    c = sc * SUPER + 4 * q
                    out_ps = pbp.tile([128, 4, DIM], F32, tag="out_ps")
                    for j in range(4):
                        # xv first (full-width, opens+zeroes the group),
                        # then the two v halves accumulate on top
                        nc.tensor.matmul(
                            out_ps[:, j, :],
                            ident_h, xv_s[:, c + j, :],
                            start=True, stop=False,
                        )
                        nc.tensor.matmul(
                            out_ps[:, j, 0:128],
                            et_s[:, c + j, 0, :], psbd[:, 0, :],
                            start=False, stop=False,
                        )
                        nc.tensor.matmul(
                            out_ps[:, j, 128:256],
                            et_s[:, c + j, 1, :], psbd[:, 1, :],
                            start=False, stop=True,
                        )
                    # fp16 downcast copy; engine alternates for balance
                    dst = ostage[:, 4 * q : 4 * q + 4, :]
                    if q % 2 == 0:
                        nc.vector.tensor_copy(dst, out_ps)
                    else:
                        nc.scalar.copy(dst, out_ps)
                nc.sync.dma_start(
                    out=o_m[:, sc * SUPER : (sc + 1) * SUPER, :], in_=ostage
                )


def fold_params(Wq, bq, K, Wv, bv, alpha, beta):
    """Host-side folding of the tiny parameter tensors (all O(256^2))."""
    import ml_dtypes

    Wq = np.asarray(Wq, np.float64)
    bq = np.asarray(bq, np.float64)
    K = np.asarray(K, np.float64)
    Wv = np.asarray(Wv, np.float64)
    bv = np.asarray(bv, np.float64)
    sa = 1.0 / (1.0 + np.exp(-np.asarray(alpha, np.float64)[:, 0]))  # (HEAD,)
    sb = 1.0 / (1.0 + np.exp(-np.asarray(beta, np.float64)[:, 0]))
    scale = 1.0 / math.sqrt(HDIM)
    # M[:, h*RANK + r] = Wq_h @ K_h^T / sqrt(d)
    M = np.zeros((DIM, HEAD * RANK))
    ab = np.zeros((HEAD * RANK,))
    for h in range(HEAD):
        Kh = K[:, h, :]  # (RANK, HDIM)
        M[:, h * RANK : (h + 1) * RANK] = (
            Wq[:, h * HDIM : (h + 1) * HDIM] @ Kh.T * scale
        )
        ab[h * RANK : (h + 1) * RANK] = (bq[h * HDIM : (h + 1) * HDIM] @ Kh.T) * scale
    sa_vec = np.repeat(sa, HDIM)  # (256,)
    swv = Wv * sa_vec[None, :]
    biasout = bv * sa_vec
    # the G accumulation carries a 64x scale (Eh' = 64*E/rs); fold the /64
    # into the per-head sig(beta) column scale
    sbcol = np.zeros((128, 2))
    for gi in range(2):
        sbcol[0:64, gi] = sb[2 * gi] / 64.0
        sbcol[64:128, gi] = sb[2 * gi + 1] / 64.0
    return {
        "mq": M.astype(ml_dtypes.float8_e4m3),
        "ab": ab.astype(np.float32),
        "swv": swv.astype(np.float16),
        "wv": Wv.astype(np.float16),
        "bv_row": (bv * 64.0).astype(np.float32).reshape(1, DIM),
        "biasout_row": biasout.astype(np.float32).reshape(1, DIM),
        "sbcol": sbcol.astype(np.float32),
    }


def build_nc(has_ab, has_bias=True):
    nc = bacc.Bacc("TRN2", target_bir_lowering=False, debug=False,
                   enable_asserts=False)
    ins = {
        "zt": nc.dram_tensor("zt", [2, 128, N], F8, kind="ExternalInput").ap(),
        "xt": nc.dram_tensor("xt", [2, 128, N], F16, kind="ExternalInput").ap(),
        "xn": nc.dram_tensor("xn", [128, NCHUNK, XW], F8, kind="ExternalInput").ap(),
        "mq": nc.dram_tensor("mq", [DIM, DIM], F8, kind="ExternalInput").ap(),
        "swv": nc.dram_tensor("swv", [DIM, DIM], F16, kind="ExternalInput").ap(),
        "wv": nc.dram_tensor("wv", [DIM, DIM], F16, kind="ExternalInput").ap(),
        "bv_row": nc.dram_tensor("bv_row", [1, DIM], F32, kind="ExternalInput").ap(),
        "biasout_row": nc.dram_tensor(
            "biasout_row", [1, DIM], F32, kind="ExternalInput"
        ).ap(),
        "sbcol": nc.dram_tensor("sbcol", [128, 2], F32, kind="ExternalInput").ap(),
        "ab_row": (
            nc.dram_tensor("ab_row", [1, DIM], F16, kind="ExternalInput").ap()
            if has_ab
            else None
        ),
    }
    ins["has_bias"] = has_bias
    outs = {"out": nc.dram_tensor("out", [N, DIM], F16, kind="ExternalOutput").ap()}
    reps = int(os.environ.get("KREPS", "1"))
    with tile.TileContext(nc) as tc:
        for _ in range(reps):
            build_body(tc, outs, ins)
    nc.compile()
    return nc


def make_core_inputs(x, z, p, has_ab):
    """Per-core DRAM input arrays from the full fp32 batch tensors."""
    import ml_dtypes

    F8NP = ml_dtypes.float8_e4m3
    common = {
        "mq": p["mq"],
        "swv": p["swv"],
        "wv": p["wv"],
        "bv_row": p["bv_row"],
        "biasout_row": p["biasout_row"],
        "sbcol": p["sbcol"],
    }
    if has_ab:
        common["ab_row"] = p["ab"].reshape(1, DIM).astype(np.float16)
    in_maps = []
    for i in range(NCORES):
        zi = np.asarray(z[i], np.float32)
        xi = np.asarray(x[i], np.float32)
        zt = np.ascontiguousarray(zi.T).astype(F8NP).reshape(2, 128, N)
        xt = np.ascontiguousarray(xi.T).astype(np.float16).reshape(2, 128, N)
        # x natural, chunk-major, padded to the aux row width so the DMA is
        # one contiguous run per partition.  aux cols: 256-259 rowsums
        # (overwritten on device), 260 esum ones, 261-263 pad.
        xn = np.zeros((128, NCHUNK, XW), F8NP)
        xn[:, :, 0:DIM] = xi.reshape(NCHUNK, 128, DIM).transpose(1, 0, 2).astype(F8NP)
        xn[:, :, DIM + 4] = F8NP(1.0)
        in_maps.append(dict(common, zt=zt, xt=xt, xn=xn))
    return in_maps


LAST_RESULTS = None


def kernel(x, z, Wq, bq, K, Wv, bv, alpha, beta):
    global LAST_RESULTS
    from concourse.bass_utils import run_bass_kernel_spmd

    x = np.asarray(x, np.float32)
    z = np.asarray(z, np.float32)
    p = fold_params(Wq, bq, K, Wv, bv, alpha, beta)
    has_ab = bool(np.any(p["ab"] != 0.0))
    has_bias = bool(np.any(p["biasout_row"] != 0.0))

    nc = build_nc(has_ab, has_bias)
    in_maps = make_core_inputs(x, z, p, has_ab)
    res = run_bass_kernel_spmd(nc, in_maps, core_ids=list(range(NCORES)))
    LAST_RESULTS = res
    out = np.stack([res.results[i]["out"] for i in range(NCORES)], axis=0)
    return out.astype(np.float32)


# revision 46
# speedup vs baseline: 1.5140x; 1.5140x over previous
# Trainium2 Bass kernel for nn_Lowrank_Spattention (sparse_attention).
#
# Reference math (per batch b, n=8192 tokens, f=256 features, h=4 heads,
# r=64 latent ranks, d=64 head dim):
#   q    = z @ Wq + bq                    (n, h*d)
#   attn = einsum(q, K)/sqrt(d)           (n, h*r)   == z @ M + ab
#            where M[:, h*r+j] = (Wq_h @ K_h^T)/8,  ab = bq @ K^T/8
#   xv   = x @ Wv + bv                    (n, h*d)
#   pooled = softmax_r(attn)^T-pool of xv (n-pool)   (r, h, d)
#   v    = softmax_n(attn) @ pooled       (n, h, d)
#   out  = sig(alpha)*xv + sig(beta)*v
#
# Kernel strategy (one NeuronCore per batch element, 8 cores, no collectives):
#   The host ships z^T (fp8), x packed (fp8), x^T (fp16), so the device
#   never transposes z or x on the PE.  The v-path's contribution to the
#   output is scaled by sig(beta)~=0.01 and pooled averages 8192 rows, so
#   its relative error is damped ~1e2-1e4x: everything E-side runs fp8
#   (DoubleRow matmuls, 2x PE throughput).  The xv-path (the dominant term)
#   is fp16 x^T against fp16 sWv.  Output is stored fp16, upcast on host.
#   Total HBM traffic 12 MB/core (was 24).
#
#   Pass A (per 128-row chunk, 4-stage pipeline over quads of chunks):
#     attn = DR(zT chunk, M)               (PE fp8 DoubleRow, one op/chunk)
#     E    = exp(attn)                     (Act->fp16; attn~N(0,1), no max sub)
#     E^T  = PE-transpose of E             (PE->PSUM, DVE copies to SBUF)
#     rs/64 = E^T @ head_indicator/64      (PE, free-dim 4: ~free)
#     rcp  = 64/rs (DVE), rs/64 -> x_s aux cols (DVE, fp8)
#     Eh'  = E * rcp  (Pool, fp8; = 64*E/rowsum)
#     G   += DR(Eh' 2-chunk, [x|aux] 2-chunk)  (PE fp8 DoubleRow, accumulate)
#   Finalize (tiny): pooled = G[:, :256] @ Wv + esum*bv;
#     PS = sig(beta)/64 * pooled / colsum, block-diagonal (fp16).
#     (colsum recovery: G[(h,r), 256+h] = sum_n Eh'*rs_fp8/64 ~= sum_n E.)
#   Pass B (per chunk): out = lhsT(xT chunk) @ sWv + lhsT(E^T) @ PS_bd
#     pure matmuls, one PSUM group per chunk, fp16 store.

import math
import os

import numpy as np

import concourse.bass as bass
import concourse.mybir as mybir
import concourse.tile as tile
from concourse import bacc

B, N, DIM = 8, 8192, 256
HEAD, RANK, HDIM = 4, 64, 64
NCORES = 8
CHUNK = 128                 # rows per compute chunk
NCHUNK = N // CHUNK         # 64
XW = DIM + 8                # x_s row width: 256 x cols + [rs0..3 | 1 | 1 1 1]
SUPER = 4                   # chunks per DMA super-chunk
NSUPER = NCHUNK // SUPER    # 8
NQ = NCHUNK // 4            # quads

F32 = mybir.dt.float32
F32R = mybir.dt.float32r
F16 = mybir.dt.float16
F8 = mybir.dt.float8e4
DR = mybir.MatmulPerfMode.DoubleRow
Exp = mybir.ActivationFunctionType.Exp


def build_body(tc, outs, ins):
    """Emit the per-core program.  outs/ins are dicts of bass.APs."""
    nc = tc.nc
    zt, xt, xn = ins["zt"], ins["xt"], ins["xn"]
    out = outs["out"]
    has_ab = ins.get("ab_row") is not None
    has_bias = bool(ins.get("has_bias", True))

    with (
        tc.tile_pool(name="consts", bufs=1) as consts,
        tc.tile_pool(name="resident", bufs=1) as resident,
    ):
        # ---- constants ----
        ident_h = consts.tile([128, 128], F16)
        nc.gpsimd.memset(ident_h, 0.0)
        nc.gpsimd.affine_select(
            out=ident_h, in_=ident_h,
            compare_op=mybir.AluOpType.not_equal, fill=1.0,
            base=0, pattern=[[-1, 128]], channel_multiplier=1,
        )
        # head indicator / 64 for PE rowsums: hind[p, t, h] = (head(t,p)==h)/64
        hind = consts.tile([128, 2, HEAD], F16)
        nc.gpsimd.memset(hind, 0.0)
        for t in range(2):
            nc.gpsimd.memset(hind[0:64, t, 2 * t : 2 * t + 1], 1.0 / 64)
            nc.gpsimd.memset(hind[64:128, t, 2 * t + 1 : 2 * t + 2], 1.0 / 64)

        mq_s = consts.tile([128, 2, DIM], F8)
        nc.sync.dma_start(out=mq_s, in_=ins["mq"].rearrange("(t p) n -> p t n", p=128))
        swv_s = consts.tile([128, 2, DIM], F16)
        nc.sync.dma_start(out=swv_s, in_=ins["swv"].rearrange("(t p) n -> p t n", p=128))
        # finalize-only consts: tiles now, DMAs issued after the pass-A loop
        # so they don't delay the first zt/xt/xn loads on the SP queue
        wv_s = consts.tile([128, 2, DIM], F16)
        sbcol_s = consts.tile([128, 2], F32)
        # broadcast rows across partitions (SWDGE replication)
        bvp_bc = consts.tile([128, DIM], F32)
        nc.gpsimd.dma_start(out=bvp_bc, in_=ins["bv_row"].to_broadcast([128, DIM]))
        biasout_bc = consts.tile([128, DIM], F32)
        nc.gpsimd.dma_start(
            out=biasout_bc, in_=ins["biasout_row"].to_broadcast([128, DIM])
        )
        if has_ab:
            ones_row = consts.tile([1, 128], F16)
            nc.vector.memset(ones_row, 1.0)
            ab_s = consts.tile([1, DIM], F16)
            nc.sync.dma_start(out=ab_s, in_=ins["ab_row"])

        # ---- residents ----
        zt_s = resident.tile([128, 2, N], F8)       # z^T (feat k-tile, n)
        xt_s = resident.tile([128, 2, N], F16)      # x^T
        x_s = resident.tile([128, NCHUNK, XW], F8)  # x natural + aux cols
        et_s = resident.tile([128, NCHUNK, 2, 128], F16)  # E^T (hr-part, rows)
        psbd = resident.tile([128, 2, 128], F16)    # block-diag PS (pass-B rhs)
        nc.gpsimd.memset(psbd, 0.0)

        # DRAM views (chunk-major row mapping: row = c*128 + p)
        zt_m = zt.rearrange("t p n -> p t n")
        xt_m = xt.rearrange("t p n -> p t n")
        o_m = out.rearrange("(c p) f -> p c f", p=128)

        with (
            tc.tile_pool(name="g_psum", bufs=1, space="PSUM") as gp,
            tc.tile_pool(name="fin_sbuf", bufs=1) as fin,
        ):
            g0 = gp.tile([128, XW], F32, tag="g0")
            g1 = gp.tile([128, XW], F32, tag="g1")

            # ================= Pass A =================
            pa_ctx = (
                tc.tile_pool(name="pa_sbuf", bufs=4),
                tc.tile_pool(name="pa_psum", bufs=2, space="PSUM"),
                tc.tile_pool(name="pa_psum1", bufs=1, space="PSUM"),
            )
            pa = pa_ctx[0].__enter__()
            pap = pa_ctx[1].__enter__()
            pap1 = pa_ctx[2].__enter__()
            e_alls, rcps = {}, {}

            def load_sc(sc):
                cols = slice(sc * SUPER * CHUNK, (sc + 1) * SUPER * CHUNK)
                nc.sync.dma_start(out=zt_s[:, :, cols], in_=zt_m[:, :, cols])
                nc.sync.dma_start(out=xt_s[:, :, cols], in_=xt_m[:, :, cols])
                nc.sync.dma_start(
                    out=x_s[:, sc * SUPER : (sc + 1) * SUPER, :],
                    in_=xn[:, sc * SUPER : (sc + 1) * SUPER, :],
                )

            load_sc(0)

            QPS = SUPER // 4   # quads per super-chunk

            def st_attn(i):
                c = 4 * i
                if (i + 1) % QPS == 0 and (i + 1) // QPS < NSUPER:
                    load_sc((i + 1) // QPS)
                attn_ps = pap.tile([128, 4, DIM], F32, tag="attn_ps")
                for j in range(4):
                    cc = slice((c + j) * CHUNK, (c + j + 1) * CHUNK)
                    nc.tensor.matmul(
                        attn_ps[:, j, :], zt_s[:, :, cc], mq_s,
                        start=True, stop=not has_ab, perf_mode=DR,
                    )
                    if has_ab:
                        nc.tensor.matmul(
                            attn_ps[:, j, :], ones_row, ab_s,
                            start=False, stop=True,
                        )
                e_all = pa.tile([128, 4, DIM], F16, tag="e_all")
                e_alls[i] = e_all
                nc.scalar.activation(e_all, attn_ps, Exp)

            def st_trans(i):
                c = 4 * i
                et_ps = pap1.tile([128, 4, 2, 128], F16, tag="et_ps")
                for j in range(4):
                    for kt in range(2):
                        nc.tensor.transpose(
                            et_ps[:, j, kt, :],
                            e_alls[i][:, j, kt * 128 : (kt + 1) * 128],
                            ident_h,
                        )
                nc.vector.tensor_copy(et_s[:, c : c + 4, :, :], et_ps)

            def st_rs(i):
                c = 4 * i
                rs_ps = pap1.tile([128, 4, HEAD], F32, tag="rs_ps")
                for j in range(4):
                    for t in range(2):
                        nc.tensor.matmul(
                            rs_ps[:, j, :],
                            et_s[:, c + j, t, :], hind[:, t, :],
                            start=(t == 0), stop=(t == 1),
                        )
                aux = bass.AP(
                    tensor=x_s.tensor,
                    offset=x_s.offset + c * XW + DIM,
                    ap=[x_s.ap[0], [XW, 4], [1, 4]],
                )
                with nc.allow_low_precision(reason="damped v-path"):
                    nc.vector.tensor_copy(aux, rs_ps)
                rcp = pa.tile([128, 4, HEAD], F32, tag="rcp")
                rcps[i] = rcp
                nc.vector.reciprocal(rcp, rs_ps)
                eh = pa.tile([128, 4, HEAD, RANK], F8, tag="eh")
                rcp_bc = bass.AP(
                    tensor=rcp.tensor,
                    offset=rcp.offset,
                    ap=[rcp.ap[0], [4, 4], [1, 4], [0, RANK]],
                )
                with nc.allow_low_precision(reason="damped v-path"):
                    nc.gpsimd.tensor_tensor(
                        out=eh,
                        in0=e_alls[i].rearrange("p c (h r) -> p c h r", h=HEAD),
                        in1=rcp_bc,
                        op=mybir.AluOpType.mult,
                    )
                return eh

            ehs = {}

            def st_g(i):
                c = 4 * i
                eh2 = ehs.pop(i).rearrange("p c h r -> p c (h r)")
                for j in (0, 2):
                    for gi, g in enumerate((g0, g1)):
                        nc.tensor.matmul(
                            g[:, 0:XW],
                            eh2[:, j : j + 2, gi * 128 : (gi + 1) * 128],
                            x_s[:, c + j : c + j + 2, :],
                            start=(c + j == 0),
                            stop=(c + j == NCHUNK - 2),
                            perf_mode=DR,
                        )

            for i in range(NQ + 3):
                if i == 3:
                    nc.sync.dma_start(
                        out=wv_s, in_=ins["wv"].rearrange("(t p) n -> p t n", p=128)
                    )
                    nc.sync.dma_start(out=sbcol_s, in_=ins["sbcol"])
                if i < NQ:
                    st_attn(i)
                if 1 <= i < NQ + 1:
                    st_trans(i - 1)
                if 2 <= i < NQ + 2:
                    ehs[i - 2] = st_rs(i - 2)
                if 3 <= i:
                    st_g(i - 3)
                    e_alls.pop(i - 3)
                    rcps.pop(i - 3)
            pa_ctx[2].__exit__(None, None, None)
            pa_ctx[1].__exit__(None, None, None)
            pa_ctx[0].__exit__(None, None, None)

            # ================= Finalize =================
            finp_ctx = tc.tile_pool(name="fin_psum", bufs=1, space="PSUM")
            finp = finp_ctx.__enter__()
            for gi, g in enumerate((g0, g1)):
                gs = fin.tile([128, XW], F16, tag=f"gs{gi}")
                nc.vector.tensor_copy(gs, g)
                gt_ps = finp.tile([128, 2, 128], F16, tag="gt_ps")
                for kt in range(2):
                    nc.tensor.transpose(
                        gt_ps[:, kt, :],
                        gs[:, kt * 128 : (kt + 1) * 128],
                        ident_h,
                    )
                gt = fin.tile([128, 2, 128], F16, tag="gt")
                nc.scalar.copy(gt, gt_ps)
                p_ps = finp.tile([128, 128], F32, tag="p_ps")
                for kt in range(2):
                    nc.tensor.matmul(
                        p_ps,
                        gt[:, kt, :],
                        wv_s[:, kt, gi * 128 : (gi + 1) * 128],
                        start=(kt == 0), stop=(kt == 1),
                    )
                # pooled = p_ps + esum * bv   (esum at aux col 260)
                pool_s = fin.tile([128, 128], F32, tag=f"pool_s{gi}")
                nc.vector.scalar_tensor_tensor(
                    out=pool_s,
                    in0=bvp_bc[:, gi * 128 : (gi + 1) * 128],
                    scalar=gs[:, DIM + 4 : DIM + 5],
                    in1=p_ps,
                    op0=mybir.AluOpType.mult,
                    op1=mybir.AluOpType.add,
                )
                # colsum (col 256+h for head h; even head rows 0:64, odd 64:128)
                cs = fin.tile([128, 1], F32, tag=f"cs{gi}")
                h0, h1 = 2 * gi, 2 * gi + 1
                nc.vector.tensor_copy(cs[0:64, :], gs[0:64, DIM + h0 : DIM + h0 + 1])
                nc.vector.tensor_copy(
                    cs[64:128, :], gs[64:128, DIM + h1 : DIM + h1 + 1]
                )
                rcs = fin.tile([128, 1], F32, tag=f"rcs{gi}")
                nc.vector.reciprocal(rcs, cs)
                nc.vector.tensor_mul(rcs, rcs, sbcol_s[:, gi : gi + 1])
                # PS block-diag (fp16): rows = this pair's (h even r | h odd r)
                nc.vector.tensor_scalar_mul(
                    psbd[0:64, gi, 0:64], pool_s[0:64, 0:64], rcs[0:64, :]
                )
                nc.vector.tensor_scalar_mul(
                    psbd[64:128, gi, 64:128], pool_s[64:128, 64:128], rcs[64:128, :]
                )

            finp_ctx.__exit__(None, None, None)

        # ================= Pass B =================
        # out = xv_s + lhsT(E^T) @ PS_bd; v-matmuls only, add + fp16 store
        with (
            tc.tile_pool(name="pb_sbuf", bufs=2) as pb,
            tc.tile_pool(name="pb_psum", bufs=4, space="PSUM") as pbp,
        ):
            for sc in range(NSUPER):
                ostage = pb.tile([128, SUPER, DIM], F16, tag="ostage")
                for q in range(SUPER // 4):
                    c = sc * SUPER + 4 * q
                    out_ps = pbp.tile([128, 4, DIM], F32, tag="out_ps")
                    for j in range(4):
                        cc = slice((c + j) * CHUNK, (c + j + 1) * CHUNK)
                        nc.tensor.matmul(
                            out_ps[:, j, :], xt_s[:, 0, cc], swv_s[:, 0, :],
                            start=True, stop=False,
                        )
                        nc.tensor.matmul(
                            out_ps[:, j, :], xt_s[:, 1, cc], swv_s[:, 1, :],
                            start=False, stop=False,
                        )
                        nc.tensor.matmul(
                            out_ps[:, j, 0:128],
                            et_s[:, c + j, 0, :], psbd[:, 0, :],
                            start=False, stop=False,
                        )
                        nc.tensor.matmul(
                            out_ps[:, j, 128:256],
                            et_s[:, c + j, 1, :], psbd[:, 1, :],
                            start=False, stop=True,
                        )
                    # out (+bias) downcast to fp16 (DVE; Act in pass B costs
                    # +10us in the scheduler model, empirically)
                    dst = ostage[:, 4 * q : 4 * q + 4, :]
                    if has_bias:
                        bias_bc4 = bass.AP(
                            tensor=biasout_bc.tensor,
                            offset=biasout_bc.offset,
                            ap=[biasout_bc.ap[0], [0, 4], [1, DIM]],
                        )
                        nc.vector.tensor_add(dst, out_ps, bias_bc4)
                    else:
                        nc.vector.tensor_copy(dst, out_ps)
                nc.sync.dma_start(
                    out=o_m[:, sc * SUPER : (sc + 1) * SUPER, :], in_=ostage
                )


def fold_params(Wq, bq, K, Wv, bv, alpha, beta):
    """Host-side folding of the tiny parameter tensors (all O(256^2))."""
    import ml_dtypes

    Wq = np.asarray(Wq, np.float64)
    bq = np.asarray(bq, np.float64)
    K = np.asarray(K, np.float64)
    Wv = np.asarray(Wv, np.float64)
    bv = np.asarray(bv, np.float64)
    sa = 1.0 / (1.0 + np.exp(-np.asarray(alpha, np.float64)[:, 0]))  # (HEAD,)
    sb = 1.0 / (1.0 + np.exp(-np.asarray(beta, np.float64)[:, 0]))
    scale = 1.0 / math.sqrt(HDIM)
    # M[:, h*RANK + r] = Wq_h @ K_h^T / sqrt(d)
    M = np.zeros((DIM, HEAD * RANK))
    ab = np.zeros((HEAD * RANK,))
    for h in range(HEAD):
        Kh = K[:, h, :]  # (RANK, HDIM)
        M[:, h * RANK : (h + 1) * RANK] = (
            Wq[:, h * HDIM : (h + 1) * HDIM] @ Kh.T * scale
        )
        ab[h * RANK : (h + 1) * RANK] = (bq[h * HDIM : (h + 1) * HDIM] @ Kh.T) * scale
    sa_vec = np.repeat(sa, HDIM)  # (256,)
    swv = Wv * sa_vec[None, :]
    biasout = bv * sa_vec
    # the G accumulation carries a 64x scale (Eh' = 64*E/rs); fold the /64
    # into the per-head sig(beta) column scale
    sbcol = np.zeros((128, 2))
    for gi in range(2):
        sbcol[0:64, gi] = sb[2 * gi] / 64.0
        sbcol[64:128, gi] = sb[2 * gi + 1] / 64.0
    return {
        "mq": M.astype(ml_dtypes.float8_e4m3),
        "ab": ab.astype(np.float32),
        "swv": swv.astype(np.float16),
        "wv": Wv.astype(np.float16),
        "bv_row": (bv * 64.0).astype(np.float32).reshape(1, DIM),
        "biasout_row": biasout.astype(np.float32).reshape(1, DIM),
        "sbcol": sbcol.astype(np.float32),
    }


def build_nc(has_ab, has_bias=True):
    nc = bacc.Bacc("TRN2", target_bir_lowering=False, debug=False,
                   enable_asserts=False)
    ins = {
        "zt": nc.dram_tensor("zt", [2, 128, N], F8, kind="ExternalInput").ap(),
        "xt": nc.dram_tensor("xt", [2, 128, N], F16, kind="ExternalInput").ap(),
        "xn": nc.dram_tensor("xn", [128, NCHUNK, XW], F8, kind="ExternalInput").ap(),
        "mq": nc.dram_tensor("mq", [DIM, DIM], F8, kind="ExternalInput").ap(),
        "swv": nc.dram_tensor("swv", [DIM, DIM], F16, kind="ExternalInput").ap(),
        "wv": nc.dram_tensor("wv", [DIM, DIM], F16, kind="ExternalInput").ap(),
        "bv_row": nc.dram_tensor("bv_row", [1, DIM], F32, kind="ExternalInput").ap(),
        "biasout_row": nc.dram_tensor(
            "biasout_row", [1, DIM], F32, kind="ExternalInput"
        ).ap(),
        "sbcol": nc.dram_tensor("sbcol", [128, 2], F32, kind="ExternalInput").ap(),
        "ab_row": (
            nc.dram_tensor("ab_row", [1, DIM], F16, kind="ExternalInput").ap()
            if has_ab
            else None
        ),
    }
    ins["has_bias"] = has_bias
    outs = {"out": nc.dram_tensor("out", [N, DIM], F16, kind="ExternalOutput").ap()}
    reps = int(os.environ.get("KREPS", "1"))
    with tile.TileContext(nc) as tc:
        for _ in range(reps):
            build_body(tc, outs, ins)
    nc.compile()
    return nc


def make_core_inputs(x, z, p, has_ab):
    """Per-core DRAM input arrays from the full fp32 batch tensors."""
    import ml_dtypes

    F8NP = ml_dtypes.float8_e4m3
    common = {
        "mq": p["mq"],
        "swv": p["swv"],
        "wv": p["wv"],
        "bv_row": p["bv_row"],
        "biasout_row": p["biasout_row"],
        "sbcol": p["sbcol"],
    }
    if has_ab:
        common["ab_row"] = p["ab"].reshape(1, DIM).astype(np.float16)
    in_maps = []
    for i in range(NCORES):
        zi = np.asarray(z[i], np.float32)
        xi = np.asarray(x[i], np.float32)
        zt = np.ascontiguousarray(zi.T).astype(F8NP).reshape(2, 128, N)
        xt = np.ascontiguousarray(xi.T).astype(np.float16).reshape(2, 128, N)
        # x natural, chunk-major, padded to the aux row width so the DMA is
        # one contiguous run per partition.  aux cols: 256-259 rowsums
        # (overwritten on device), 260 esum ones, 261-263 pad.
        xn = np.zeros((128, NCHUNK, XW), F8NP)
        xn[:, :, 0:DIM] = xi.reshape(NCHUNK, 128, DIM).transpose(1, 0, 2).astype(F8NP)
        xn[:, :, DIM + 4] = F8NP(1.0)
        in_maps.append(dict(common, zt=zt, xt=xt, xn=xn))
    return in_maps


LAST_RESULTS = None


def kernel(x, z, Wq, bq, K, Wv, bv, alpha, beta):
    global LAST_RESULTS
    from concourse.bass_utils import run_bass_kernel_spmd

    x = np.asarray(x, np.float32)
    z = np.asarray(z, np.float32)
    p = fold_params(Wq, bq, K, Wv, bv, alpha, beta)
    has_ab = bool(np.any(p["ab"] != 0.0))
    has_bias = bool(np.any(p["biasout_row"] != 0.0))

    nc = build_nc(has_ab, has_bias)
    in_maps = make_core_inputs(x, z, p, has_ab)
    res = run_bass_kernel_spmd(nc, in_maps, core_ids=list(range(NCORES)))
    LAST_RESULTS = res
    out = np.stack([res.results[i]["out"] for i in range(NCORES)], axis=0)
    return out.astype(np.float32)
